# revision 17
# baseline (speedup 1.0000x reference)
"""KANvolution Trainium2 Bass kernel (v2: hat basis + bf16 + col-tiled PE).

Math: per patch element x and per (f,c,ki,kj):
    K(x) = w_spline * sum_g hat_g(clip(x)) * cp_g  +  w_silu * silu(x)
with hat_g the normalized linear B-spline (tent) basis on the 17-knot
grid in [-1,1] (hat sum == 1, so the reference's /(sum+1e-8) is a
constant 1/(1+1e-8) scale folded into the weights).

Instead of the relu-telescope decomposition (dense features, poorly
conditioned in bf16), we evaluate the tent basis directly:
    nhat_g(x) = min(|8*clip(x) - k_g| - 1, 0)   ( = -hat_g, k_g = 8*g_g )
and negate the spline weights host-side.  Only 2 of 17 hats are nonzero
per element and |v| ~ 1e-2, so bf16 matmul error is negligible.

Per tap (ki,kj) the contraction is 17 hats x 32 ch (+ silu x 32 + bias)
= 5 k-tiles of <=128.  9 taps x 5 k-tiles x 4 row-chunks = 180 matmuls
of [K<=128, N=64] x [K, M=512] per core, all bf16.

PE col-tiling: the F=64 output only fills half the 128-wide PE array,
so matmuls alternate between array column groups 0-63 / 64-127
(tile_position inferred from the PSUM slice base partition).  The two
groups stream concurrently through separate XBUSes -> ~2x throughput.
Each PSUM bank holds two independent accumulators (partitions 0-63 and
64-127); both halves are copied to SBUF, DMAed out, and summed on the
host.

Sharding: 8 cores = (batch b, output-row half).  Each core gets a
(34, 66, 32) input slab pre-transposed and pre-scaled (8x, bf16) on the
host and produces [128, 32*64] partial outputs (two 64-filter halves).
"""

import numpy as np
from contextlib import ExitStack

import concourse.bacc as bacc
import concourse.mybir as mybir
import concourse.tile as tile
from concourse.bass_utils import run_bass_kernel_spmd

# Problem constants (hardcoded per harness contract)
B, H, W, C, F = 4, 66, 66, 32, 64
KH = KW = 3
G = 16                                   # spline intervals; G+1 = 17 knots
HO, WO = H - KH + 1, W - KW + 1          # 64, 64
N_CORES = 8
ROWS_PER_CORE = HO // 2                  # 32 output rows
IN_ROWS = ROWS_PER_CORE + KH - 1         # 34 input rows
SPAT = IN_ROWS * W                       # 2244 input spatial positions
SPAT_PAD = 2304                          # pad to 18*128
N_TAPS = KH * KW                         # 9
N_KTILES = 5                             # 4 hat tiles (4x32 rows) + tail tile
K_TAIL = 65                              # tail tile rows: hat16(32)+silu(32)+bias(1)
CHUNK_ROWS = 8                           # output rows per matmul chunk
N_CHUNKS = ROWS_PER_CORE // CHUNK_ROWS   # 4
NFREE = CHUNK_ROWS * WO                  # 512 moving-dim per matmul
SLABS = [(0, 660), (660, 1188), (1188, 2304)]   # feature column slabs
N_WARMUP = 8                             # HAM warm-up matmuls

_COMPILED = None  # cached (nc) program


def _build_weights(control_points, w_spline, w_silu, bias):
    """Host-side transform of KAN params into the [128, 45*64] bf16 matrix.

    Column block (t*9 + tap)*64 .. +64 holds k-tile t of tap (ki,kj):
      t<4 : row r*32+c = knot g=4t+r, channel c, value -v[f,c,ki,kj,g]
      t=4 : rows 0-31 = knot 16 (negated), rows 32-63 = w_silu,
            row 64 = bias (tap 0 only).
    Hat weights are negated because the kernel computes -hat.
    """
    import ml_dtypes
    cp = control_points.astype(np.float64)
    ws = w_spline.astype(np.float64)
    v = ws[..., None] * cp / (1.0 + 1e-8)          # (F, C, 3, 3, 17)

    w_all = np.zeros((N_KTILES, N_TAPS, 128, F), dtype=np.float64)
    for i in range(KH):
        for j in range(KW):
            tap = i * KW + j
            for t in range(4):
                for r in range(4):
                    g = 4 * t + r
                    w_all[t, tap, r * 32:(r + 1) * 32, :] = -v[:, :, i, j, g].T
            w_all[4, tap, 0:32, :] = -v[:, :, i, j, 16].T
            w_all[4, tap, 32:64, :] = w_silu[:, :, i, j].astype(np.float64).T
    w_all[4, 0, 64, :] = bias.astype(np.float64)
    w_host = w_all.transpose(2, 0, 1, 3).reshape(128, N_KTILES * N_TAPS * F)
    return np.ascontiguousarray(w_host.astype(ml_dtypes.bfloat16))


def _build_program():
    nc = bacc.Bacc("TRN2", target_bir_lowering=False, debug=False,
                   num_devices=N_CORES)
    f32 = mybir.dt.float32
    bf16 = mybir.dt.bfloat16
    AF = mybir.ActivationFunctionType
    OP = mybir.AluOpType
    import os
    # CoreSim has no Silu; swap in Sigmoid for sim-only structure checks.
    AF_SILU = AF.Sigmoid if os.environ.get("KAN_SIM_SAFE") else AF.Silu

    x_in = nc.declare_dram_parameter("x8t", [32, SPAT_PAD], bf16, isOutput=False)
    w_in = nc.declare_dram_parameter("w", [128, N_KTILES * N_TAPS * F], bf16,
                                     isOutput=False)
    kv_in = nc.declare_dram_parameter("kv", [128, 8], f32, isOutput=False)
    ones_in = nc.declare_dram_parameter("ones", [1, SPAT_PAD], bf16,
                                        isOutput=False)
    y_out = nc.declare_dram_parameter("y", [128, N_CHUNKS * NFREE], f32,
                                      isOutput=True)

    with tile.TileContext(nc) as tc:
        with ExitStack() as ctx:
            sb = ctx.enter_context(tc.tile_pool(name="sb", bufs=1))
            ps = ctx.enter_context(tc.tile_pool(name="ps", bufs=1, space="PSUM"))
            ob = ctx.enter_context(tc.tile_pool(name="ob", bufs=2))

            # --- static inputs (spread across engine DMA queues; x first) ---
            x_rep = sb.tile([128, SPAT_PAD], bf16, tag="xrep")
            kv_sb = sb.tile([128, 8], f32, tag="kv")
            nc.sync.dma_start(kv_sb[:], kv_in[:])
            qs = [nc.sync, nc.gpsimd]
            for gg in range(4):                  # x, replicated x4 on partitions
                qs[gg % 2].dma_start(x_rep[32 * gg:32 * (gg + 1), :], x_in[:])
            w_sb = sb.tile([128, N_KTILES * N_TAPS * F], bf16, tag="w")
            # t=0 block first (first-needed), remainder as one big transfer
            nc.sync.dma_start(w_sb[:, 0:576], w_in[:, 0:576])
            nc.gpsimd.dma_start(w_sb[:, 576:2880], w_in[:, 576:2880])

            # feature tiles
            xc8 = sb.tile([128, SPAT_PAD], bf16, tag="xc8")
            tb = [sb.tile([128, SPAT_PAD], bf16, name=f"tb{u}", tag=f"tb{u}")
                  for u in range(2)]
            nhat = [sb.tile([128, SPAT_PAD], bf16, name=f"nh{t}", tag=f"nh{t}")
                    for t in range(N_KTILES)]
            nc.sync.dma_start(nhat[4][64:65, :], ones_in[:])   # bias row = 1.0

            # warm the ACT table set (silu's set; relu/abs/copy are fillers)
            warm = sb.tile([1, 8], f32, tag="warm")
            nc.scalar.activation(warm[:], kv_sb[0:1, :], AF_SILU)


            def features(t):
                """All three column slabs of k-tile t (t-major emission)."""
                for a, b in SLABS:
                    cs = slice(a, b)
                    if t == 0:  # xc8 = clip(8x) to [-8, 8]
                        nc.vector.tensor_scalar(xc8[:, cs], x_rep[:, cs],
                                                8.0, -8.0, OP.min, OP.max)
                    if t < 4:
                        tbt = tb[t % 2]
                        # |xc8 - k| on ACT (per-partition bias = -k)
                        nc.scalar.activation(tbt[:, cs], xc8[:, cs], AF.Abs,
                                             bias=kv_sb[:, t:t + 1], scale=1.0)
                        nc.vector.tensor_scalar(nhat[t][:, cs], tbt[:, cs],
                                                1.0, 0.0, OP.subtract, OP.min)
                    else:       # tail: hat16 (rows 0-31), silu (rows 32-63)
                        tbt = tb[0]
                        nc.scalar.activation(tbt[0:32, cs], xc8[0:32, cs],
                                             AF.Abs, bias=kv_sb[0:32, 4:5],
                                             scale=1.0)
                        nc.vector.tensor_scalar(nhat[4][0:32, cs],
                                                tbt[0:32, cs],
                                                1.0, 0.0, OP.subtract, OP.min)
                        nc.scalar.activation(nhat[4][32:64, cs],
                                             x_rep[32:64, cs],
                                             AF_SILU, scale=0.125)

            # Two PSUM banks per chunk: col-group A (array cols 0-63) in bank
            # 0 / partitions 0-63, group B in bank 1 / partitions 64-127 —
            # one accumulation group per zero region.  All 4 chunks stay
            # resident so each weight load serves 4 matmuls.
            P = [ps.tile([128, 2 * NFREE], f32, name=f"po{q}", tag=f"po{q}")
                 for q in range(N_CHUNKS)]

            # HAM warm-up: junk matmuls on the x tile keep PE busy while
            # features are computed, so real matmuls run at full clock.
            # They write P[0]'s A-region with start/stop pairs; the first
            # real matmul's start=True clears it.
            for u in range(N_WARMUP):
                nc.tensor.matmul(P[0][0:F, 0:NFREE], x_rep[:, 0:F],
                                 x_rep[:, 0:NFREE], start=True, stop=True)
            order = [(t, tap) for t in range(N_KTILES) for tap in range(N_TAPS)]
            grp_of = [n % 2 for n in range(len(order))]
            last = {}
            for n, g_ in enumerate(grp_of):
                last[g_] = n
            started = set()
            for n, (t, tap) in enumerate(order):
                if tap == 0:
                    features(t)
                g_ = grp_of[n]
                i, j = divmod(tap, KW)
                kk = 128 if t < 4 else K_TAIL
                col = (t * N_TAPS + tap) * F
                lhsT = w_sb[0:kk, col:col + F]
                for q in range(N_CHUNKS):
                    base = (CHUNK_ROWS * q + i) * W
                    rhs = (nhat[t][0:kk, base:base + CHUNK_ROWS * W]
                           .rearrange("p (r w) -> p r w", w=W)
                           [:, :, j:j + WO])
                    nc.tensor.matmul(
                        P[q][F * g_:F * (g_ + 1), NFREE * g_:NFREE * (g_ + 1)]
                            .rearrange("f (r w) -> f r w", w=WO),
                        lhsT, rhs,
                        start=((q, g_) not in started),
                        stop=(n == last[g_]),
                    )
                    started.add((q, g_))

            # PSUM -> SBUF (halves stay separate; host adds them; bias rides
            # on the A-half copy) and DMA out, spread over engine queues.
            for q in range(N_CHUNKS):
                stage = ob.tile([128, NFREE], f32, tag="stage")
                nc.scalar.copy(stage[0:F, :], P[q][0:F, 0:NFREE])
                nc.vector.tensor_copy(stage[F:128, :],
                                      P[q][F:128, NFREE:2 * NFREE])
                qs[q % 2].dma_start(y_out[:, NFREE * q:NFREE * (q + 1)],
                                    stage[:])

    nc.compile()
    return nc


def _get_program():
    global _COMPILED
    if _COMPILED is None:
        _COMPILED = _build_program()
    return _COMPILED


def _make_in_maps(x, control_points, w_spline, w_silu, bias):
    import ml_dtypes
    bf = ml_dtypes.bfloat16
    w_host = _build_weights(control_points, w_spline, w_silu, bias)
    # ACT Abs bias: tb = Abs(xc8 + kv[:,t]) with kv = -(knot) = 8 - (4t + g)
    kv = np.zeros((128, 8), dtype=np.float32)
    for t in range(4):
        for p in range(128):
            kv[p, t] = 8.0 - (4 * t + p // 32)
    kv[:, 4] = -8.0                       # tail-tile (knot 16) Abs bias
    ones = np.ones((1, SPAT_PAD), dtype=bf)

    x8 = (np.asarray(x, dtype=np.float32) * 8.0).astype(bf)
    in_maps = []
    for core in range(N_CORES):
        b, half = divmod(core, 2)
        r0 = half * ROWS_PER_CORE
        xs = np.zeros((32, SPAT_PAD), dtype=bf)
        xs[:, :SPAT] = x8[b, r0:r0 + IN_ROWS].reshape(SPAT, C).T
        in_maps.append({"x8t": xs, "w": w_host, "kv": kv, "ones": ones})
    return in_maps


def kernel(x, control_points, w_spline, w_silu, bias):
    in_maps = _make_in_maps(x, control_points, w_spline, w_silu, bias)
    nc = _get_program()
    res = run_bass_kernel_spmd(nc, in_maps, list(range(N_CORES)))

    out = np.empty((B, HO, WO, F), dtype=np.float32)
    for core in range(N_CORES):
        b, half = divmod(core, 2)
        r0 = half * ROWS_PER_CORE
        y2 = res.results[core]["y"]                    # [128, 2048]
        y = y2[0:F] + y2[F:128]                        # [64, 2048]
        out[b, r0:r0 + ROWS_PER_CORE] = (
            y.reshape(F, ROWS_PER_CORE, WO).transpose(1, 2, 0))
    return out
